# revision 45
# baseline (speedup 1.0000x reference)
"""GCNRouting2Hop on 8 trn2 NeuronCores (Bass/Tile SPMD kernel).

Sharding: dst-node partition (2500 nodes/core, 20 blocks of 128).
Self-loops are folded into the edge list (norm = dinv^2) on host.

Layer 1 has NO dynamic gathers: the gather indices are host-known, so x
is pre-expanded into a per-edge stream xg [128, CH, 128] bf16 (like the
one-hot tiles) and streamed on the SP HWDGE queue while one-hot scatter
tiles stream on the ACT HWDGE queue. Per chunk the TensorEngine
accumulates zT += xg.T @ onehot in PSUM; dense matmuls + LN follow.

h is cast to fp8e4 and AllGathered in 3 slices (triggered during L1
from the otherwise-idle gpsimd queue; fewer rendezvous pipelines the
chain better than finer slicing). Layer 2 gathers h rows in fp8 (256B
descriptors, 2x the drain rate of bf16) via 2 sub-gathers per block:
sub-A (sources in AG slices 0-1, in_ap = hg_full[0:16384]) and sub-B
(slice-2 sources, in_ap = hg_full[16384:]), so Tile gates each on only
the AG slices it needs. The L2 scatter runs in the zT domain
(lhsT=gh_half fp8 x rhs=oht bf16 mixed matmul -> [feat,dst] PSUM), so
no transposes; W2 halves then contract [feat,dst].T @ W2 into
[dst,256] with bias and the identity-residual, then LayerNorm.
"""
import os
import sys
import types

sys.path.insert(0, '/opt/trn_rl_repo')
import numpy as np


def _install_axon_hooks_shim():
    try:
        import antenv
    except ImportError:
        return
    if hasattr(antenv, 'axon_hooks') or 'antenv.axon_hooks' in sys.modules:
        return
    try:
        from trn_agent_boot.trn_boot import _ntff_profile_via_ctypes
        hook = _ntff_profile_via_ctypes('/opt/axon/libaxon_pjrt.so')
    except Exception:
        hook = None
    mod = types.ModuleType('antenv.axon_hooks')
    mod._hook = hook
    mod.get_axon_ntff_profile_hook = lambda: mod._hook

    def set_axon_ntff_profile_hook(h):
        mod._hook = h

    mod.set_axon_ntff_profile_hook = set_axon_ntff_profile_hook
    sys.modules['antenv.axon_hooks'] = mod
    antenv.axon_hooks = mod


_install_axon_hooks_shim()

import ml_dtypes
from concourse import bacc, mybir, tile
from concourse.masks import make_identity
from concourse.bass_utils import run_bass_kernel_spmd

N = 20000
NC = 8
NPC = N // NC              # 2500 dst nodes per core
NBLK = (NPC + 127) // 128  # 20 blocks of 128 dst nodes
DIN = 128
DH = 256
LN_EPS = 1e-5

# AllGather slice layout: 3 slices of local rows per core
SLQ = (1024, 1024, NPC - 2048)           # 1024,1024,452
SLO = (0, 1024, 2048)
SL_LAST_BLOCK = (7, 15, 19)              # trigger slice q after this block
A_ROWS = NC * 2048                        # hg_full rows covered by slices 0-1

LAST_EXEC_TIME_NS = None
_prog_cache = {}

f32 = mybir.dt.float32
bf16 = mybir.dt.bfloat16
fp8 = mybir.dt.float8e4
i16 = mybir.dt.int16
i32 = mybir.dt.int32

# cstf fp32 [128, 4, 256] rows; cstb bf16 [128, 7, 256]
(F_G1, F_BE1, F_G2, F_BE2) = range(4)
(B_W1, B_W2A, B_W2B, B_WRES, B_B1, B_BRES, B_B2) = range(7)


def _ln(nc, epi, u, gt, bt, out_tile, eps_ap):
    """LayerNorm over free axis; nodes on partitions. DVE kept off the
    slow TensorScalarPtr path: reductions on ACT accumulators, the
    normalize on ACT Identity with per-partition scale/bias."""
    sq = epi.tile([128, DH], f32, tag="sq")
    s1 = epi.tile([128, 1], f32, tag="s1")
    s2 = epi.tile([128, 1], f32, tag="s2")
    nc.scalar.activation(sq[:], u[:], mybir.ActivationFunctionType.Copy,
                         accum_out=s1[:])
    nc.scalar.activation(sq[:], u[:], mybir.ActivationFunctionType.Square,
                         accum_out=s2[:])
    mu = epi.tile([128, 1], f32, tag="mu")
    nc.vector.tensor_scalar(out=mu[:], in0=s1[:], scalar1=1.0 / DH,
                            scalar2=None, op0=mybir.AluOpType.mult)
    var = epi.tile([128, 1], f32, tag="var")
    musq = epi.tile([128, 1], f32, tag="musq")
    nc.vector.tensor_tensor(out=musq[:], in0=mu[:], in1=mu[:],
                            op=mybir.AluOpType.mult)
    nc.vector.tensor_scalar(out=var[:], in0=s2[:], scalar1=1.0 / DH,
                            scalar2=None, op0=mybir.AluOpType.mult)
    nc.vector.tensor_tensor(out=var[:], in0=var[:], in1=musq[:],
                            op=mybir.AluOpType.subtract)
    std = epi.tile([128, 1], f32, tag="std")
    nc.scalar.activation(std[:], var[:], mybir.ActivationFunctionType.Sqrt,
                         bias=eps_ap)
    rstd = epi.tile([128, 1], f32, tag="rstd")
    nc.vector.reciprocal(rstd[:], std[:])
    nmr = epi.tile([128, 1], f32, tag="nmr")
    nc.vector.tensor_tensor(out=nmr[:], in0=mu[:], in1=rstd[:],
                            op=mybir.AluOpType.mult)
    nc.vector.tensor_scalar(out=nmr[:], in0=nmr[:], scalar1=-1.0,
                            scalar2=None, op0=mybir.AluOpType.mult)
    un = epi.tile([128, DH], f32, tag="un")
    nc.scalar.activation(un[:], u[:], mybir.ActivationFunctionType.Identity,
                         bias=nmr[:], scale=rstd[:])
    g = epi.tile([128, DH], f32, tag="g")
    nc.vector.tensor_tensor(out=g[:], in0=un[:], in1=gt,
                            op=mybir.AluOpType.mult)
    nc.vector.tensor_tensor(out=out_tile[:], in0=g[:], in1=bt,
                            op=mybir.AluOpType.add)


def _build_program(padA, padB, zr, hgdt_name, shared):
    hgdt = {"fp8": fp8, "bf16": bf16}[hgdt_name]
    R = tuple(a + b for a, b in zip(padA, padB))
    offs = np.concatenate([[0], np.cumsum(R)]).astype(np.int64)
    E_pad = int(offs[-1])
    CH = E_pad // 128
    smax = int(max(R)) // 128

    nc = bacc.Bacc("TRN2", target_bir_lowering=False, debug=False,
                   num_devices=NC, num_swdge_queues=4)
    xg_in = nc.dram_tensor("xg", [128, CH, DIN], bf16, kind="ExternalInput")
    oh_in = nc.dram_tensor("ohb", [128, CH, 128], bf16, kind="ExternalInput")
    idx2_in = nc.dram_tensor("idx2", [128, E_pad // 16], i16,
                             kind="ExternalInput")
    xT_in = nc.dram_tensor("xT", [128, NBLK * 128], bf16,
                           kind="ExternalInput")
    cstf_in = nc.dram_tensor("cstf", [128, 4, DH], f32, kind="ExternalInput")
    cstb_in = nc.dram_tensor("cstb", [128, 7, DH], bf16, kind="ExternalInput")
    cnt_in = nc.dram_tensor("cnt2", [1, NBLK * 2], i32, kind="ExternalInput")
    out_t = nc.dram_tensor("out", [NPC, DH], f32, kind="ExternalOutput")

    with tile.TileContext(nc) as tc:
        with tc.tile_pool(name="keep", bufs=1) as keep, \
             tc.tile_pool(name="xgp", bufs=7) as xgp, \
             tc.tile_pool(name="ohp", bufs=7) as ohp, \
             tc.tile_pool(name="oh2p", bufs=4) as oh2p, \
             tc.tile_pool(name="ghp", bufs=5) as ghp, \
             tc.tile_pool(name="rot", bufs=3) as rot, \
             tc.tile_pool(name="epi", bufs=3) as epi, \
             tc.tile_pool(name="ps_dn", bufs=4, space="PSUM") as ps_dn, \
             tc.tile_pool(name="ps_ag", bufs=4, space="PSUM") as ps_ag, \
             tc.tile_pool(name="dram", bufs=1, space="DRAM") as dram:

            # ---- preload ----
            # scalar (ACT HWDGE) queue: small consts needed by block-0
            # epilogue, then the one-hot streams share this queue.
            cstf = keep.tile([128, 4, DH], f32)
            nc.scalar.dma_start(cstf[:], cstf_in[:])
            cstb = keep.tile([128, 7, DH], bf16)
            nc.scalar.dma_start(cstb[:], cstb_in[:])
            # gpsimd queue (idle until L2): L2-only / residual inputs.
            cnt_t = keep.tile([1, NBLK * 2], i32)
            nc.gpsimd.dma_start(cnt_t[:], cnt_in[:])
            xT = keep.tile([128, NBLK * 128], bf16)
            nc.gpsimd.dma_start(xT[:], xT_in[:])
            idx2 = keep.tile([128, E_pad // 16], i16)
            nc.gpsimd.dma_start(idx2[:], idx2_in[:])
            # TensorLoad caps at 32 registers per instruction; split in two
            _, cnt_lo = nc.values_load_multi_w_load_instructions(
                cnt_t[:, 0:NBLK], engines=(mybir.EngineType.Pool,),
                min_val=0, max_val=int(max(R)),
                skip_runtime_bounds_check=True)
            _, cnt_hi = nc.values_load_multi_w_load_instructions(
                cnt_t[:, NBLK:NBLK * 2], engines=(mybir.EngineType.Pool,),
                min_val=0, max_val=int(max(R)),
                skip_runtime_bounds_check=True)
            cnt_vals = tuple(cnt_lo) + tuple(cnt_hi)

            # Pre-zero the at-risk chunk ranges of each rotating gh slot
            # (tail-skipped pad rows leave bytes unwritten; uninitialized
            # SBUF can be NaN and NaN*0 poisons PSUM).
            for _i in range(5):
                k0, k1 = zr[_i]
                _hz = ghp.tile([128, smax, DH], hgdt, tag="gh")
                if k1 > k0:
                    nc.vector.memset(_hz[:, k0:k1, :], 0)

            eps_t = keep.tile([128, 1], f32)
            nc.vector.memset(eps_t[:], LN_EPS)
            ones_t = keep.tile([1, 128], bf16)
            nc.vector.memset(ones_t[:], 1.0)
            ident = keep.tile([128, 128], bf16)
            make_identity(nc, ident[:])
            h_own = keep.tile([128, NBLK * DH], bf16)

            hg_self = dram.tile([NPC, DH], hgdt)
            if shared:
                hg_full = dram.tile([N, DH], hgdt, addr_space="Shared")
            else:
                hg_full = dram.tile([N, DH], hgdt)

            g1t = cstf[:, F_G1, :]
            be1t = cstf[:, F_BE1, :]
            g2t = cstf[:, F_G2, :]
            be2t = cstf[:, F_BE2, :]
            b1row = cstb[0:1, B_B1, :]
            bresrow = cstb[0:1, B_BRES, :]
            b2row = cstb[0:1, B_B2, :]
            W1b = cstb[:, B_W1, :]
            W2ab = cstb[:, B_W2A, :]
            W2bb = cstb[:, B_W2B, :]
            Wresb = cstb[:, B_WRES, :]

            # ---- layer 1 (no gathers: xg + oht streamed) ----
            # Streams are issued P blocks ahead so the oht issue on the
            # in-order ACT queue is not stuck behind the previous block's
            # LN work (which itself waits on that block's matmuls).
            P = 6
            xg_t = {}
            oh_t = {}

            def issue_l1(b):
                nchunk = R[b] // 128
                t0 = int(offs[b]) // 128
                xg_t[b] = xgp.tile([128, smax, DIN], bf16, tag="xg",
                                   name=f"xg{b}")
                nc.sync.dma_start(xg_t[b][:, 0:nchunk, :],
                                  xg_in[:, t0:t0 + nchunk, :])
                oh_t[b] = ohp.tile([128, smax, 128], bf16, tag="oh1",
                                   name=f"oh{b}")
                nc.scalar.dma_start(oh_t[b][:, 0:nchunk, :],
                                    oh_in[:, t0:t0 + nchunk, :])

            for b in range(P):
                issue_l1(b)
            for b in range(NBLK):
                if b + P < NBLK:
                    issue_l1(b + P)
                nchunk = R[b] // 128
                xgt = xg_t.pop(b)
                oht = oh_t.pop(b)
                psum_zT = ps_ag.tile([128, 128], f32, tag="agg",
                                     space="PSUM")
                for k in range(nchunk):
                    nc.tensor.matmul(out=psum_zT[:], lhsT=xgt[:, k, :],
                                     rhs=oht[:, k, :], start=(k == 0),
                                     stop=(k == nchunk - 1))
                zts = rot.tile([128, 128], bf16, tag="zts")
                nc.scalar.activation(zts[:], psum_zT[:],
                                     mybir.ActivationFunctionType.Copy)
                psum_h1 = ps_dn.tile([128, DH], f32, tag="dense",
                                     space="PSUM")
                nc.tensor.matmul(out=psum_h1[:], lhsT=ones_t[:], rhs=b1row,
                                 start=True, stop=False)
                nc.tensor.matmul(out=psum_h1[:], lhsT=zts[:], rhs=W1b,
                                 start=False, stop=True)
                psum_r = ps_dn.tile([128, DH], f32, tag="dense", space="PSUM")
                nc.tensor.matmul(out=psum_r[:], lhsT=ones_t[:], rhs=bresrow,
                                 start=True, stop=False)
                nc.tensor.matmul(out=psum_r[:],
                                 lhsT=xT[:, b * 128:(b + 1) * 128],
                                 rhs=Wresb, start=False, stop=True)
                delta = epi.tile([128, DH], f32, tag="delta")
                nc.scalar.activation(delta[:], psum_h1[:],
                                     mybir.ActivationFunctionType.Relu)
                u = epi.tile([128, DH], f32, tag="u")
                nc.vector.tensor_tensor(out=u[:], in0=psum_r[:],
                                        in1=delta[:], op=mybir.AluOpType.add)
                hblk = h_own[:, b * DH:(b + 1) * DH]
                _ln(nc, epi, u, g1t, be1t, hblk, eps_t[:])
                hsb = rot.tile([128, DH], hgdt, tag="hsb")
                nc.scalar.activation(hsb[:], hblk,
                                     mybir.ActivationFunctionType.Copy)
                rows = min(128, NPC - b * 128)
                # store on the ACT queue right after the cast executes
                nc.scalar.dma_start(
                    out=hg_self[b * 128:b * 128 + rows, :],
                    in_=hsb[0:rows, :])
                if b in SL_LAST_BLOCK:
                    q = SL_LAST_BLOCK.index(b)
                    lo = SLO[q]
                    base = NC * SLO[q]
                    nc.gpsimd.collective_compute(
                        "AllGather", mybir.AluOpType.bypass,
                        replica_groups=[list(range(NC))],
                        ins=[hg_self[lo:lo + SLQ[q], :]],
                        outs=[hg_full[base:base + NC * SLQ[q], :]])

            # ---- layer 2 ----
            oh2_t = {}

            def issue_l2_oh(b):
                nchunk = R[b] // 128
                t0 = int(offs[b]) // 128
                oh2_t[b] = oh2p.tile([128, smax, 128], bf16, tag="oh2",
                                     name=f"oh2_{b}")
                nc.scalar.dma_start(oh2_t[b][:, 0:nchunk, :],
                                    oh_in[:, t0:t0 + nchunk, :])

            for b in range(3):
                issue_l2_oh(b)
            for b in range(NBLK):
                if b + 3 < NBLK:
                    issue_l2_oh(b + 3)
                nchunk = R[b] // 128
                kA = padA[b] // 128
                o16 = int(offs[b]) // 16
                oht2 = oh2_t.pop(b)
                gh = ghp.tile([128, smax, DH], hgdt, tag="gh")
                nc.gpsimd.dma_gather(
                    out_ap=gh[:, 0:kA, :], in_ap=hg_full[0:A_ROWS, :],
                    idxs_ap=idx2[:, o16:o16 + padA[b] // 16],
                    num_idxs=padA[b], num_idxs_reg=cnt_vals[2 * b],
                    elem_size=DH, single_packet=False, queue_num=b % 4)
                if padB[b] > 0:
                    oB = o16 + padA[b] // 16
                    nc.gpsimd.dma_gather(
                        out_ap=gh[:, kA:nchunk, :],
                        in_ap=hg_full[A_ROWS:N, :],
                        idxs_ap=idx2[:, oB:oB + padB[b] // 16],
                        num_idxs=padB[b], num_idxs_reg=cnt_vals[2 * b + 1],
                        elem_size=DH, single_packet=False,
                        queue_num=(b + 2) % 4)
                psum_a = ps_ag.tile([128, 128], f32, tag="agg", space="PSUM")
                psum_b = ps_ag.tile([128, 128], f32, tag="agg", space="PSUM")
                for k in range(nchunk):
                    nc.tensor.matmul(out=psum_a[:], lhsT=gh[:, k, 0:128],
                                     rhs=oht2[:, k, :], start=(k == 0),
                                     stop=(k == nchunk - 1))
                    nc.tensor.matmul(out=psum_b[:], lhsT=gh[:, k, 128:256],
                                     rhs=oht2[:, k, :], start=(k == 0),
                                     stop=(k == nchunk - 1))
                za = rot.tile([128, 128], bf16, tag="za")
                nc.scalar.activation(za[:], psum_a[:],
                                     mybir.ActivationFunctionType.Copy)
                zb = rot.tile([128, 128], bf16, tag="zb")
                nc.scalar.activation(zb[:], psum_b[:],
                                     mybir.ActivationFunctionType.Copy)
                psum_d2 = ps_dn.tile([128, DH], f32, tag="dense",
                                     space="PSUM")
                nc.tensor.matmul(out=psum_d2[:], lhsT=ones_t[:], rhs=b2row,
                                 start=True, stop=False)
                nc.tensor.matmul(out=psum_d2[:], lhsT=za[:], rhs=W2ab,
                                 start=False, stop=False)
                nc.tensor.matmul(out=psum_d2[:], lhsT=zb[:], rhs=W2bb,
                                 start=False, stop=False)
                nc.tensor.matmul(out=psum_d2[:], lhsT=ident[:],
                                 rhs=h_own[:, b * DH:(b + 1) * DH],
                                 start=False, stop=True)
                outb = epi.tile([128, DH], f32, tag="outb")
                _ln(nc, epi, psum_d2, g2t, be2t, outb, eps_t[:])
                rows = min(128, NPC - b * 128)
                nc.sync.dma_start(out=out_t[b * 128:b * 128 + rows, :],
                                  in_=outb[0:rows, :])
    nc.compile()
    return nc


def _host_prep(edge_index, edge_weight):
    """Edge preprocessing: self-loops folded in, per-(core, block) edge
    lists sorted so sources in AG slices 0-3 (sub-A) precede slice-4
    sources (sub-B), each sub-range padded to a chunk multiple."""
    src = np.asarray(edge_index[0], np.int64)
    dst = np.asarray(edge_index[1], np.int64)
    w = np.asarray(edge_weight, np.float32)
    deg = np.ones(N, np.float32)  # self-loop weight 1 included
    np.add.at(deg, dst, w)
    dinv = np.where(deg > 0, 1.0 / np.sqrt(deg), 0.0).astype(np.float32)
    loop = np.arange(N, dtype=np.int64)
    src_a = np.concatenate([src, loop])
    dst_a = np.concatenate([dst, loop])
    norm_a = np.concatenate([(dinv[src] * w * dinv[dst]).astype(np.float32),
                             (dinv * dinv).astype(np.float32)])

    # sub-B edges are those whose source sits in AG slice 4
    is_b = (src_a % NPC) >= 2048
    # sort by (dst block, is_b) so each block's sub-A edges come first
    bkey = (dst_a // NPC) * NBLK + (dst_a % NPC) // 128
    order = np.lexsort((is_b, bkey))
    src_s, dst_s, norm_s, isb_s = (src_a[order], dst_a[order],
                                   norm_a[order], is_b[order])
    bkey_s = bkey[order]

    core_id = dst_s // NPC
    brel = (dst_s % NPC) // 128
    cntA = np.zeros((NC, NBLK), np.int64)
    cntB = np.zeros((NC, NBLK), np.int64)
    np.add.at(cntA, (core_id[~isb_s], brel[~isb_s]), 1)
    np.add.at(cntB, (core_id[isb_s], brel[isb_s]), 1)
    padA = tuple(int(v) for v in
                 (np.ceil(cntA.max(axis=0) / 128) * 128).astype(np.int64))
    padB = tuple(int(v) for v in
                 (np.ceil(cntB.max(axis=0) / 128) * 128).astype(np.int64))
    R = tuple(a + b for a, b in zip(padA, padB))
    offs = np.concatenate([[0], np.cumsum(R)]).astype(np.int64)
    E_pad = int(offs[-1])

    src_pad = np.full((NC, E_pad), -1, np.int64)
    dstrel_pad = np.zeros((NC, E_pad), np.int64)
    wn_pad = np.zeros((NC, E_pad), np.float32)
    real = np.zeros((NC, E_pad), bool)
    # block starts in the sorted edge array (bkey_s is monotonic)
    blk_lo = np.searchsorted(bkey_s, np.arange(NC * NBLK), 'left')
    for c in range(NC):
        for b in range(NBLK):
            i = c * NBLK + b
            lo = blk_lo[i]
            hi = blk_lo[i + 1] if i + 1 < NC * NBLK else len(dst_s)
            nA = int(cntA[c, b])
            nB = int(cntB[c, b])
            o = int(offs[b])
            oB = o + padA[b]
            # sub-A edges are the first nA of the block (lexsort)
            src_pad[c, o:o + nA] = src_s[lo:lo + nA]
            dstrel_pad[c, o:o + nA] = (dst_s[lo:lo + nA]
                                       - (c * NPC + b * 128))
            wn_pad[c, o:o + nA] = norm_s[lo:lo + nA]
            real[c, o:o + nA] = True
            src_pad[c, oB:oB + nB] = src_s[lo + nA:hi]
            dstrel_pad[c, oB:oB + nB] = (dst_s[lo + nA:hi]
                                         - (c * NPC + b * 128))
            wn_pad[c, oB:oB + nB] = norm_s[lo + nA:hi]
            real[c, oB:oB + nB] = True
    return (padA, padB, cntA.astype(np.int32), cntB.astype(np.int32),
            src_pad, dstrel_pad, wn_pad, real)


def _rowmap():
    """Global node id -> row in the slice-interleaved hg_full layout."""
    n_all = np.arange(N, dtype=np.int64)
    c_all = n_all // NPC
    l_all = n_all % NPC
    q_all = np.minimum(l_all // 1024, 2)
    slq = np.asarray(SLQ, np.int64)
    slo = np.asarray(SLO, np.int64)
    base = np.concatenate([[0], np.cumsum(NC * slq)])[:-1]
    return (base[q_all] + c_all * slq[q_all]
            + (l_all - slo[q_all])).astype(np.int64)


def kernel(x, edge_index, edge_weight, W1, b1, W2, b2, Wres, bres,
           gamma1, beta1, gamma2, beta2):
    global LAST_EXEC_TIME_NS
    x = np.ascontiguousarray(np.asarray(x, np.float32))
    W1 = np.asarray(W1, np.float32)
    W2 = np.asarray(W2, np.float32)
    Wres = np.asarray(Wres, np.float32)

    hgdt_name = os.environ.get("GCN_HG_DT", "fp8")
    shared = os.environ.get("GCN_SHARED", "0") == "1"

    (padA, padB, cntA, cntB, src_pad, dstrel_pad, wn_pad,
     real) = _host_prep(edge_index, edge_weight)
    R = tuple(a + b for a, b in zip(padA, padB))
    offs = np.concatenate([[0], np.cumsum(R)]).astype(np.int64)
    E_pad = int(offs[-1])
    CH = E_pad // 128

    cstf = np.zeros((128, 4, DH), np.float32)
    cstf[:, F_G1, :] = np.asarray(gamma1, np.float32)[None, :]
    cstf[:, F_BE1, :] = np.asarray(beta1, np.float32)[None, :]
    cstf[:, F_G2, :] = np.asarray(gamma2, np.float32)[None, :]
    cstf[:, F_BE2, :] = np.asarray(beta2, np.float32)[None, :]
    cstb = np.zeros((128, 7, DH), np.float32)
    cstb[:, B_W1, :] = W1
    cstb[:, B_W2A, :] = W2[:128, :]
    cstb[:, B_W2B, :] = W2[128:, :]
    cstb[:, B_WRES, :] = Wres
    cstb[:, B_B1, :] = np.asarray(b1, np.float32)[None, :]
    cstb[:, B_BRES, :] = np.asarray(bres, np.float32)[None, :]
    cstb[:, B_B2, :] = np.asarray(b2, np.float32)[None, :]
    cstb = cstb.astype(ml_dtypes.bfloat16)

    xbf = x.astype(ml_dtypes.bfloat16)
    rowmap = _rowmap()

    in_maps = []
    for c in range(NC):
        sp = src_pad[c]
        valid = sp >= 0
        # idx2: hg_full rows; sub-B indices are relative to hg_full[A_ROWS:]
        r_abs = np.where(valid, rowmap[np.where(valid, sp, 0)], -1)
        isB = np.zeros(E_pad, bool)
        for b in range(NBLK):
            isB[int(offs[b]) + padA[b]:int(offs[b + 1])] = True
        idx2_flat = np.where(valid, r_abs - np.where(isB, A_ROWS, 0),
                             -1).astype(np.int16)
        idx2_w = np.tile(idx2_flat.reshape(E_pad // 16, 16).T, (8, 1)).copy()
        # per-edge x stream [128, CH, 128] bf16 (pads are zero rows)
        xg = np.zeros((E_pad, DIN), ml_dtypes.bfloat16)
        xg[valid] = xbf[sp[valid]]
        xg = np.ascontiguousarray(
            xg.reshape(CH, 128, DIN).transpose(1, 0, 2))
        # one-hot scatter tiles [128, CH, 128]
        oh = np.zeros((E_pad, 128), np.float32)
        rr = real[c]
        oh[np.nonzero(rr)[0], dstrel_pad[c][rr]] = wn_pad[c][rr]
        oh = np.ascontiguousarray(oh.reshape(CH, 128, 128).transpose(1, 0, 2))
        ohb = oh.astype(ml_dtypes.bfloat16)
        xT = np.zeros((128, NBLK * 128), np.float32)
        xT[:, :NPC] = x[c * NPC:(c + 1) * NPC].T
        cnt2 = np.empty((1, NBLK * 2), np.int32)
        cnt2[0, 0::2] = cntA[c]
        cnt2[0, 1::2] = cntB[c]
        in_maps.append({
            "xg": xg,
            "ohb": ohb,
            "idx2": idx2_w,
            "xT": xT.astype(ml_dtypes.bfloat16),
            "cstf": cstf,
            "cstb": cstb,
            "cnt2": cnt2,
        })

    # per-slot zero regions for the rotating gh slots: union of risky
    # chunk ranges (pad tails of sub-A and sub-B) over blocks b==s (mod 5)
    minA = cntA.min(axis=0)
    zr = []
    for s in range(5):
        bs = [b for b in range(NBLK) if b % 5 == s]
        k0 = min(int(minA[b]) // 128 for b in bs)
        k1 = max(R[b] // 128 for b in bs)
        zr.append((k0, k1))
    zr = tuple(zr)

    key = (padA, padB, zr, hgdt_name, shared)
    nc = _prog_cache.get(key)
    if nc is None:
        nc = _build_program(padA, padB, zr, hgdt_name, shared)
        _prog_cache[key] = nc

    trace = bool(os.environ.get("BASS_KERNEL_TRACE"))
    res = run_bass_kernel_spmd(nc, in_maps, list(range(NC)), trace=trace)
    if trace:
        LAST_EXEC_TIME_NS = res.exec_time_ns
    out = np.concatenate([res.results[c]["out"] for c in range(NC)], axis=0)
    return np.ascontiguousarray(out.astype(np.float32))
